# revision 5
# baseline (speedup 1.0000x reference)
"""Trainium2 Bass kernel for the dense branch-MLP problem.

Computes: out[b,o] = sum_n relu((s[b,:] - v[n,:]) @ W[n].T + bias[n])[o]
with B=1024, N=64, D=512, OUT=2048 in fp32.

Sharding: expert-style across the N=64 branch axis -> 8 branches per core.
Each core computes a full [B, OUT] partial sum over its 8 branches; the
host sums the 8 partials (the unshard step).

Per-core kernel (PE-bound, ~17.2 GFLOP at fp32r rates):
  - s^T resident in SBUF as 4 d-chunks [128, 1024]
  - per branch: offs = s^T - v_n (ScalarE, per-partition bias), stream
    W[n]^T tiles as matmul stationary operands, accumulate over d in PSUM,
    relu+bias on ScalarE, branch-sum on VectorE.
  - matmuls run in float32r (fp22 internal) at 1 cycle/row since the
    moving free dim is 512.
"""

import numpy as np

import concourse.bacc as bacc
import concourse.mybir as mybir
import concourse.tile as tile
from concourse.bass_utils import run_bass_kernel_spmd

B, N, D, OUT = 1024, 64, 512, 2048
N_CORES = 8
NL = N // N_CORES  # branches per core
DC = D // 128  # d chunks (4)
OT = OUT // 128  # o tiles (16)
BT = B // 512  # b free-dim tiles (2)

F32 = mybir.dt.float32
F32R = mybir.dt.float32r
RELU = mybir.ActivationFunctionType.Relu
IDENT = mybir.ActivationFunctionType.Identity

_cache = {}


def build(repeat: int = 1):
    """Build + compile the per-core Bass program. Cached per `repeat`."""
    if repeat in _cache:
        return _cache[repeat]

    nc = bacc.Bacc(
        "TRN2",
        target_bir_lowering=False,
        debug=False,
        num_devices=N_CORES,
    )

    wt_d = nc.dram_tensor("wt", [NL, 128, DC * OUT], F32R, kind="ExternalInput").ap()
    st_d = nc.dram_tensor("st", [128, DC * B], F32, kind="ExternalInput").ap()
    negv_d = nc.dram_tensor("negv", [128, NL * DC], F32, kind="ExternalInput").ap()
    bias_d = nc.dram_tensor("bias", [128, NL * OT], F32, kind="ExternalInput").ap()
    out_d = nc.dram_tensor("out", [OUT, B], F32, kind="ExternalOutput").ap()

    with tile.TileContext(nc) as tc:
        with (
            tc.tile_pool(name="const", bufs=1) as const_pool,
            tc.tile_pool(name="acc", bufs=1) as acc_pool,
            tc.tile_pool(name="offs", bufs=2) as offs_pool,
            tc.tile_pool(name="wt", bufs=2) as wt_pool,
            tc.tile_pool(name="tmp", bufs=4) as tmp_pool,
            tc.tile_pool(name="psum", bufs=6, space="PSUM") as psum_pool,
        ):
            st = const_pool.tile([128, DC * B], F32, name="st")
            nc.sync.dma_start(st[:], st_d[:])
            negv = const_pool.tile([128, NL * DC], F32, name="negv")
            nc.sync.dma_start(negv[:], negv_d[:])
            bias = const_pool.tile([128, NL * OT], F32, name="bias")
            nc.sync.dma_start(bias[:], bias_d[:])

            acc = [
                acc_pool.tile([128, B], F32, name=f"acc{ot}", tag=f"acc{ot}")
                for ot in range(OT)
            ]

            def body(iv=None):
                for n in range(NL):
                    wt = wt_pool.tile([128, DC * OUT], F32R, name="wt_t", tag="wt_t")
                    nc.sync.dma_start(wt[:], wt_d[n])

                    offs = offs_pool.tile([128, DC * B], F32R, name="offs", tag="offs")
                    for c in range(DC):
                        nc.scalar.activation(
                            offs[:, c * B : (c + 1) * B],
                            st[:, c * B : (c + 1) * B],
                            IDENT,
                            bias=negv[:, n * DC + c : n * DC + c + 1],
                            scale=1.0,
                        )

                    for ot in range(OT):
                        for bt in range(BT):
                            ps = psum_pool.tile([128, 512], F32, name="ps", tag="ps")
                            for c in range(DC):
                                nc.tensor.matmul(
                                    ps[:],
                                    wt[:, c * OUT + ot * 128 : c * OUT + (ot + 1) * 128],
                                    offs[:, c * B + bt * 512 : c * B + bt * 512 + 512],
                                    start=(c == 0),
                                    stop=(c == DC - 1),
                                )
                            b_ap = bias[:, n * OT + ot : n * OT + ot + 1]
                            if n == 0:
                                nc.scalar.activation(
                                    acc[ot][:, bt * 512 : bt * 512 + 512],
                                    ps[:],
                                    RELU,
                                    bias=b_ap,
                                    scale=1.0,
                                )
                            else:
                                tmp = tmp_pool.tile([128, 512], F32, name="tmp", tag="tmp")
                                nc.scalar.activation(tmp[:], ps[:], RELU, bias=b_ap, scale=1.0)
                                nc.vector.tensor_add(
                                    acc[ot][:, bt * 512 : bt * 512 + 512],
                                    acc[ot][:, bt * 512 : bt * 512 + 512],
                                    tmp[:],
                                )

                for ot in range(OT):
                    nc.sync.dma_start(out_d[ot * 128 : (ot + 1) * 128, :], acc[ot][:])

            if repeat == 1:
                body()
            else:
                with tc.For_i(0, repeat, 1):
                    body()

    nc.compile()
    _cache[repeat] = nc
    return nc


def prep_inputs(semantic_vec, vertices, W, b):
    """Host-side layout transforms -> per-core input maps."""
    semantic_vec = np.asarray(semantic_vec, dtype=np.float32)
    vertices = np.asarray(vertices, dtype=np.float32)
    W = np.asarray(W, dtype=np.float32)
    b = np.asarray(b, dtype=np.float32)

    # st[p, c*B + bb] = s[bb, c*128+p]
    st = np.ascontiguousarray(
        semantic_vec.reshape(B, DC, 128).transpose(2, 1, 0).reshape(128, DC * B)
    )
    # wt[n, p, c*OUT + o] = W[n, o, c*128+p]
    wt = np.ascontiguousarray(
        W.reshape(N, OUT, DC, 128).transpose(0, 3, 2, 1).reshape(N, 128, DC * OUT)
    )
    # negv[p, nl*DC + c] = -v[n0+nl, c*128+p]
    negv = np.ascontiguousarray(
        (-vertices).reshape(N_CORES, NL, DC, 128).transpose(0, 3, 1, 2).reshape(N_CORES, 128, NL * DC)
    )
    # bias[p, nl*OT + ot] = b[n0+nl, ot*128+p]
    bias = np.ascontiguousarray(
        b.reshape(N_CORES, NL, OT, 128).transpose(0, 3, 1, 2).reshape(N_CORES, 128, NL * OT)
    )

    in_maps = []
    for core in range(N_CORES):
        in_maps.append(
            {
                "wt": wt[core * NL : (core + 1) * NL],
                "st": st,
                "negv": negv[core],
                "bias": bias[core],
            }
        )
    return in_maps


def kernel(semantic_vec, vertices, W, b):
    nc = build(repeat=1)
    in_maps = prep_inputs(semantic_vec, vertices, W, b)
    res = run_bass_kernel_spmd(nc, in_maps, core_ids=list(range(N_CORES)))
    total = np.zeros((OUT, B), dtype=np.float32)
    for core in range(N_CORES):
        total += res.results[core]["out"]
    return np.ascontiguousarray(total.T)
